# revision 5
# baseline (speedup 1.0000x reference)
"""Trainium2 Bass kernel for nn_Net_16174846837292 (NNConv GNN message passing).

Strategy (graph-sharded, aggregation-folded):
  pooled[g,o] = sum_{e: batch[dst[e]]=g} w_e * msg[e,o],  w_e = 1/max(cnt[dst_e],1)
  msg[e,o]    = sum_{k,i} e3[e,k]*h[src_e,i]*e4w[k,i*128+o] + sum_i h[src_e,i]*e4b[i*128+o]
  => pooled^T[o,g] = sum_k A2[k*128+i, o]^T ZG_g[i,k] + Br^T HW_g        (tiny matmuls)
     ZG_g[i,k] = sum_{e in g} w_e h[src_e,i] e3[e,k],  HW_g[i] = sum_{e in g} w_e h[src_e,i]
  This never materializes the per-edge [128,128] weight matrices (512 MB in the
  reference) nor any per-node [16384] intermediates.

Sharding: edges are grouped by the graph of their destination node; 8 graphs per
core (64 graphs / 8 cores). Each graph's incoming edges live entirely on one
core, so per-node in-degree counts are core-local and NO collectives are needed.
Each core's edges are packed into 8 slots of 2x128-edge tiles (max 169 edges per
graph in this dataset). Node MLP is replicated (cheap); per-core output is its 8
graphs' pooled rows; host concatenates.
"""

import os
import numpy as np
from contextlib import ExitStack

import concourse.bass as bass
import concourse.tile as tile
from concourse import bacc, mybir
from concourse.bass_utils import run_bass_kernel_spmd

N_CORES = 8
N, E, G, H = 4096, 8192, 64, 128
NODE_DIM, EDGE_DIM = 11, 5
G_PER_CORE = G // N_CORES          # 8 graph slots per core
SLOT_TILES = 2                     # 128-edge tiles per slot
CAP = SLOT_TILES * 128             # 256 edge slots per graph
EP = G_PER_CORE * CAP              # 2048 edge slots per core
NT = EP // 128                     # 16 edge tiles per core
NB = 768                           # histogram bins; bin NB-1 collects padding
NBC = NB // 128                    # 6 bin chunks
NCH = EP // 512                    # 4 512-wide chunks for the edge MLP

f32 = mybir.dt.float32
i32 = mybir.dt.int32
AF = mybir.ActivationFunctionType
OP = mybir.AluOpType


def _emit(nc, tc, io):
    ctx = tc and None  # unused
    x_t, ea_t = io["xT"], io["eaT"]
    p1w, p1b, p2w, p2b = io["p1w"], io["p1b"], io["p2w"], io["p2b"]
    e1w, e1b, e2w, e2b = io["e1w"], io["e1b"], io["e2w"], io["e2b"]
    e3w, e3b = io["e3w"], io["e3b"]
    a2, br, ident = io["a2"], io["br"], io["ident"]
    srcidx, dstl, winv, maskf = io["srcidx"], io["dstl"], io["winv"], io["mask"]
    pooled_t = io["pooled_t"]

    es = ExitStack()
    const = es.enter_context(tc.tile_pool(name="const", bufs=1))
    big = es.enter_context(tc.tile_pool(name="big", bufs=1))
    work = es.enter_context(tc.tile_pool(name="work", bufs=3))
    e3x = es.enter_context(tc.tile_pool(name="e3x", bufs=NT))
    psA = es.enter_context(tc.tile_pool(name="psA", bufs=2, space="PSUM"))
    psB = es.enter_context(tc.tile_pool(name="psB", bufs=3, space="PSUM"))
    psZ = es.enter_context(tc.tile_pool(name="psZ", bufs=2, space="PSUM"))
    psO = es.enter_context(tc.tile_pool(name="psO", bufs=1, space="PSUM"))
    dram = es.enter_context(tc.tile_pool(name="dram", bufs=1, space="DRAM"))

    with es:
        # ---- constant loads -------------------------------------------------
        def cload(name, ap, shape, dt=f32):
            t = const.tile(shape, dt, tag=name)
            nc.sync.dma_start(t[:], ap[:])
            return t

        xT = cload("xT", x_t, [NODE_DIM, N])
        idn = cload("ident", ident, [128, 128])
        w_p1 = cload("p1w", p1w, [NODE_DIM, H])
        b_p1 = cload("p1b", p1b, [H, 1])
        w_p2 = cload("p2w", p2w, [H, H])
        b_p2 = cload("p2b", p2b, [H, 1])
        w_e1 = cload("e1w", e1w, [EDGE_DIM, 128])
        b_e1 = cload("e1b", e1b, [128, 1])
        w_e2 = cload("e2w", e2w, [128, 256])
        b_e2 = cload("e2b", e2b, [128, 2])
        w_e30 = const.tile([128, 128], f32, tag="e3w0")
        nc.sync.dma_start(w_e30[:], e3w[0:128, :])
        w_e31 = const.tile([128, 128], f32, tag="e3w1")
        nc.sync.dma_start(w_e31[:], e3w[128:256, :])
        b_e3 = cload("e3b", e3b, [128, 1])
        w_br = cload("br", br, [H, H])
        eaT = cload("eaT", ea_t, [EDGE_DIM, EP])
        t_src = cload("srcidx", srcidx, [128, NT], i32)
        t_dstl = cload("dstl", dstl, [128, NT])
        t_winv = cload("winv", winv, [128, NT], i32)
        t_mask = cload("mask", maskf, [128, NT])
        # big A2 (8 MB) on SWDGE so the HWDGE queue stays free for small loads
        a2_sb = big.tile([128, 128 * H], f32, tag="a2")
        nc.gpsimd.dma_start(a2_sb[:], a2[:])

        h_dram = dram.tile([N, H], f32)
        inv_dram = dram.tile([NB, 1], f32)

        # ---- node MLP (feature-major), h rows to DRAM -----------------------
        for c in range(N // 512):
            ps1 = psA.tile([128, 512], f32, tag="mlp")
            nc.tensor.matmul(ps1[:], w_p1[:], xT[:, c * 512:(c + 1) * 512],
                             start=True, stop=True)
            h1 = work.tile([128, 512], f32, tag="h1")
            nc.scalar.activation(h1[:], ps1[:], AF.Relu, bias=b_p1[:, 0:1])
            ps2 = psA.tile([128, 512], f32, tag="mlp")
            nc.tensor.matmul(ps2[:], w_p2[:], h1[:], start=True, stop=True)
            h2 = work.tile([128, 512], f32, tag="h2")
            nc.vector.tensor_scalar_add(h2[:], ps2[:], b_p2[:, 0:1])
            for j in range(4):
                pt = psB.tile([128, 128], f32, tag="tr")
                nc.tensor.transpose(pt[:], h2[:, j * 128:(j + 1) * 128], idn[:])
                hr = work.tile([128, 128], f32, tag="hrow")
                nc.scalar.copy(hr[:], pt[:])
                nc.sync.dma_start(
                    h_dram[c * 512 + j * 128: c * 512 + (j + 1) * 128, :], hr[:])

        # ---- in-degree histogram (DVE) & 1/max(cnt,1) -----------------------
        ioti = const.tile([128, NB], i32, tag="iotai")
        nc.gpsimd.iota(ioti[:], pattern=[[1, NB]], base=0, channel_multiplier=0)
        iot = const.tile([128, NB], f32, tag="iota")
        nc.vector.tensor_copy(iot[:], ioti[:])
        oh_acc = big.tile([128, NB], f32, tag="ohacc")
        nc.vector.tensor_scalar(oh_acc[:], iot[:], t_dstl[:, 0:1], None,
                                op0=OP.is_equal)
        for t in range(1, NT):
            oh = work.tile([128, NB], f32, tag="oh")
            nc.vector.tensor_scalar(oh[:], iot[:], t_dstl[:, t:t + 1], None,
                                    op0=OP.is_equal)
            nc.vector.tensor_add(oh_acc[:], oh_acc[:], oh[:])
        cnt = work.tile([128, NBC], f32, tag="cnt")
        for b in range(NBC):
            pt = psB.tile([128, 128], f32, tag="tr")
            nc.tensor.transpose(pt[:], oh_acc[:, b * 128:(b + 1) * 128], idn[:])
            nc.vector.tensor_reduce(cnt[:, b:b + 1], pt[:],
                                    axis=mybir.AxisListType.X, op=OP.add)
        nc.vector.tensor_scalar_max(cnt[:], cnt[:], 1.0)
        inv = work.tile([128, NBC], f32, tag="inv")
        nc.vector.reciprocal(inv[:], cnt[:])
        nc.sync.dma_start(inv_dram[:, 0].rearrange("(p c) -> p c", c=NBC), inv[:])
        # per-edge weights: gather 1/cnt by dst, then mask out padding
        wg = work.tile([128, NT], f32, tag="wg")
        for t in range(NT):
            nc.gpsimd.indirect_dma_start(
                out=wg[:, t:t + 1], out_offset=None, in_=inv_dram[:, :],
                in_offset=bass.IndirectOffsetOnAxis(ap=t_winv[:, t:t + 1], axis=0))
        wme = const.tile([128, NT], f32, tag="wme")
        nc.vector.tensor_mul(wme[:], wg[:], t_mask[:])

        # ---- edge MLP (feature-major) ---------------------------------------
        e1o = big.tile([128, EP], f32, tag="e1o")
        for q in range(NCH):
            ps = psA.tile([128, 512], f32, tag="mlp")
            nc.tensor.matmul(ps[:], w_e1[:], eaT[:, q * 512:(q + 1) * 512],
                             start=True, stop=True)
            nc.scalar.activation(e1o[:, q * 512:(q + 1) * 512], ps[:], AF.Relu,
                                 bias=b_e1[:, 0:1])
        e2o0 = big.tile([128, EP], f32, tag="e2o0")
        e2o1 = big.tile([128, EP], f32, tag="e2o1")
        for m, e2o in enumerate((e2o0, e2o1)):
            for q in range(NCH):
                ps = psA.tile([128, 512], f32, tag="mlp")
                nc.tensor.matmul(ps[:], w_e2[:, m * 128:(m + 1) * 128],
                                 e1o[:, q * 512:(q + 1) * 512],
                                 start=True, stop=True)
                nc.scalar.activation(e2o[:, q * 512:(q + 1) * 512], ps[:],
                                     AF.Relu, bias=b_e2[:, m:m + 1])
        e3o = big.tile([128, EP], f32, tag="e3o")
        for q in range(NCH):
            ps = psA.tile([128, 512], f32, tag="mlp")
            nc.tensor.matmul(ps[:], w_e30[:], e2o0[:, q * 512:(q + 1) * 512],
                             start=True, stop=False)
            nc.tensor.matmul(ps[:], w_e31[:], e2o1[:, q * 512:(q + 1) * 512],
                             start=False, stop=True)
            nc.scalar.activation(e3o[:, q * 512:(q + 1) * 512], ps[:], AF.Relu,
                                 bias=b_e3[:, 0:1])

        # ---- per-tile transpose to edge-major, scale by w_e -----------------
        e3w_tiles = []
        for t in range(NT):
            pt = psB.tile([128, 128], f32, tag="tr")
            nc.tensor.transpose(pt[:], e3o[:, t * 128:(t + 1) * 128], idn[:])
            ex = e3x.tile([128, H + 1], f32, tag="e3x")
            nc.vector.tensor_scalar_mul(ex[:, 0:H], pt[:], wme[:, t:t + 1])
            nc.vector.tensor_copy(ex[:, H:H + 1], wme[:, t:t + 1])
            e3w_tiles.append(ex)

        # ---- gather h[src] rows (edge-major) --------------------------------
        hsrc = big.tile([128, EP], f32, tag="hsrc")
        for t in range(NT):
            nc.gpsimd.indirect_dma_start(
                out=hsrc[:, t * 128:(t + 1) * 128], out_offset=None,
                in_=h_dram[:, :],
                in_offset=bass.IndirectOffsetOnAxis(ap=t_src[:, t:t + 1], axis=0))

        # ---- per-graph ZG accumulation: [i, k..128] per slot ----------------
        zg = big.tile([128, G_PER_CORE, H + 1], f32, tag="zg")
        for s in range(G_PER_CORE):
            pz = psZ.tile([128, H + 1], f32, tag="zg")
            for tt in range(SLOT_TILES):
                t = s * SLOT_TILES + tt
                nc.tensor.matmul(pz[:], hsrc[:, t * 128:(t + 1) * 128],
                                 e3w_tiles[t][:],
                                 start=(tt == 0), stop=(tt == SLOT_TILES - 1))
            nc.scalar.copy(zg[:, s, :], pz[:])

        # ---- final contraction with e4 --------------------------------------
        po = psO.tile([128, G_PER_CORE], f32, tag="out")
        for k in range(H):
            nc.tensor.matmul(po[:], a2_sb[:, k * 128:(k + 1) * 128],
                             zg[:, :, k], start=(k == 0), stop=False)
        nc.tensor.matmul(po[:], w_br[:], zg[:, :, H], start=False, stop=True)
        ot = work.tile([128, G_PER_CORE], f32, tag="ot")
        nc.scalar.copy(ot[:], po[:])
        nc.sync.dma_start(pooled_t[:, :], ot[:])


_CACHE = {}


def _build():
    if "nc" in _CACHE:
        return _CACHE["nc"]
    nc = bacc.Bacc("TRN2", target_bir_lowering=False, debug=False,
                   num_devices=N_CORES)
    io = {}

    def din(name, shape, dt=f32):
        io[name] = nc.dram_tensor(name, shape, dt, kind="ExternalInput").ap()

    din("xT", [NODE_DIM, N])
    din("eaT", [EDGE_DIM, EP])
    din("srcidx", [128, NT], i32)
    din("dstl", [128, NT])
    din("winv", [128, NT], i32)
    din("mask", [128, NT])
    din("p1w", [NODE_DIM, H]); din("p1b", [H, 1])
    din("p2w", [H, H]); din("p2b", [H, 1])
    din("e1w", [EDGE_DIM, 128]); din("e1b", [128, 1])
    din("e2w", [128, 256]); din("e2b", [128, 2])
    din("e3w", [256, 128]); din("e3b", [128, 1])
    din("a2", [128, 128 * H])
    din("br", [H, H])
    din("ident", [128, 128])
    io["pooled_t"] = nc.dram_tensor("pooled_t", [H, G_PER_CORE], f32,
                                    kind="ExternalOutput").ap()

    with tile.TileContext(nc) as tc:
        _emit(nc, tc, io)
    nc.compile()
    _CACHE["nc"] = nc
    return nc


def _host_prep(inputs):
    x = np.ascontiguousarray(np.asarray(inputs["x"], dtype=np.float32))
    ea = np.asarray(inputs["edge_attr"], dtype=np.float32)
    ei = np.asarray(inputs["edge_index"]).astype(np.int64)
    batch = np.asarray(inputs["batch"]).astype(np.int64)
    src, dst = ei[0], ei[1]
    gid = batch[dst]

    com = {
        "xT": np.ascontiguousarray(x.T),
        "p1w": np.asarray(inputs["p1_w"], np.float32),
        "p1b": np.asarray(inputs["p1_b"], np.float32).reshape(H, 1),
        "p2w": np.asarray(inputs["p2_w"], np.float32),
        "p2b": np.asarray(inputs["p2_b"], np.float32).reshape(H, 1),
        "e1w": np.asarray(inputs["e1_w"], np.float32),
        "e1b": np.asarray(inputs["e1_b"], np.float32).reshape(128, 1),
        "e2w": np.asarray(inputs["e2_w"], np.float32),
        "e2b": np.ascontiguousarray(
            np.asarray(inputs["e2_b"], np.float32).reshape(2, 128).T),
        "e3w": np.asarray(inputs["e3_w"], np.float32),
        "e3b": np.asarray(inputs["e3_b"], np.float32).reshape(128, 1),
        "a2": np.ascontiguousarray(
            np.asarray(inputs["e4_w"], np.float32)
            .reshape(128, 128, 128).transpose(1, 0, 2).reshape(128, 128 * H)),
        "br": np.ascontiguousarray(
            np.asarray(inputs["e4_b"], np.float32).reshape(128, 128)),
        "ident": np.eye(128, dtype=np.float32),
    }
    com = {k: np.ascontiguousarray(v) for k, v in com.items()}

    # per-core node ranges (batch is sorted)
    ns = np.searchsorted(batch, np.arange(0, G + 1, G_PER_CORE))
    in_maps = []
    for c in range(N_CORES):
        n0, n1 = int(ns[c]), int(ns[c + 1])
        assert n1 - n0 <= NB - 2, f"core {c} has {n1 - n0} nodes > {NB - 2}"
        ea_s = np.zeros((EP, EDGE_DIM), np.float32)
        src_s = np.zeros(EP, np.int32)
        dstl_s = np.full(EP, NB - 1, np.int32)
        mask_s = np.zeros(EP, np.float32)
        for s in range(G_PER_CORE):
            es = np.where(gid == c * G_PER_CORE + s)[0]
            assert len(es) <= CAP, f"graph {c * G_PER_CORE + s}: {len(es)} edges"
            pos = s * CAP + np.arange(len(es))
            ea_s[pos] = ea[es]
            src_s[pos] = src[es]
            dstl_s[pos] = dst[es] - n0
            mask_s[pos] = 1.0
        winv_s = (dstl_s % 128) * NBC + dstl_s // 128

        def grid(a):  # slot pos = t*128 + p  ->  [p, t]
            return np.ascontiguousarray(a.reshape(NT, 128).T)

        m = dict(com)
        m["eaT"] = np.ascontiguousarray(ea_s.T)
        m["srcidx"] = grid(src_s)
        m["dstl"] = grid(dstl_s).astype(np.float32)
        m["winv"] = grid(winv_s)
        m["mask"] = grid(mask_s)
        in_maps.append(m)
    return in_maps


def _run(inputs, trace=False, tmpdir=None):
    nc = _build()
    in_maps = _host_prep(inputs)
    if trace:
        # No egress in this sandbox: neutralize the artifact upload the
        # trace path performs after NTFF capture, and register the NTFF
        # hook module if the image lacks antenv.axon_hooks.
        from concourse import bass_utils as _bu
        _bu.upload_artifacts = lambda d: d
        try:
            from antenv import axon_hooks  # noqa: F401
        except ImportError:
            import importlib.util, sys as _sys
            spec = importlib.util.spec_from_file_location(
                "antenv.axon_hooks", "/opt/trn_rl_repo/antenv/axon_hooks.py")
            mod = importlib.util.module_from_spec(spec)
            spec.loader.exec_module(mod)
            _sys.modules["antenv.axon_hooks"] = mod
    res = run_bass_kernel_spmd(nc, in_maps, list(range(N_CORES)),
                               trace=trace, tmpdir=tmpdir)
    out = np.empty((G, H), np.float32)
    for c in range(N_CORES):
        out[c * G_PER_CORE:(c + 1) * G_PER_CORE, :] = res.results[c]["pooled_t"].T
    return out, res


def kernel(**inputs) -> np.ndarray:
    out, _ = _run(inputs)
    return out


# revision 12
# speedup vs baseline: 1.6599x; 1.6599x over previous
"""Trainium2 Bass kernel for nn_Net_16174846837292 (NNConv GNN message passing).

Strategy (graph-sharded, aggregation-folded):
  pooled[g,o] = sum_{e: batch[dst[e]]=g} w_e * msg[e,o],  w_e = 1/max(cnt[dst_e],1)
  msg[e,o]    = sum_{k,i} e3[e,k]*h[src_e,i]*e4w[k,i*128+o] + sum_i h[src_e,i]*e4b[i*128+o]
  => pooled^T[o,g] = sum_k A2[k*128+i, o]^T ZG_g[i,k] + Br^T HW_g        (tiny matmuls)
     ZG_g[i,k] = sum_{e in g} w_e h[src_e,i] e3[e,k],  HW_g[i] = sum_{e in g} w_e h[src_e,i]
  Never materializes the per-edge [128,128] weight matrices (512 MB in the
  reference) nor any per-node [16384] intermediates.

Sharding: edges grouped by the graph of their destination node; 8 graphs per
core. Each graph's incoming edges live entirely on one core, so in-degree
counts are core-local and NO collectives are needed. Per-core edges pack into
8 slots of 192 (64-aligned segments for the per-graph PSUM accumulation).
Node MLP runs only over the <=1280 unique src nodes each core references.
The e4 contraction runs as a 3-term bf16 split (zh@ah + zl@ah + zh@al, fp32
PSUM) — max error vs fp32 measured at 2.9e-6 of output scale.
"""

import numpy as np
from contextlib import ExitStack

import ml_dtypes
import concourse.bass as bass
import concourse.tile as tile
from concourse import bacc, mybir
from concourse.bass_utils import run_bass_kernel_spmd

N_CORES = 8
N, E, G, H = 4096, 8192, 64, 128
NODE_DIM, EDGE_DIM = 11, 5
G_PER_CORE = G // N_CORES          # 8 graph slots per core
CAP = 192                          # edge slots per graph (64-aligned segments)
EP = G_PER_CORE * CAP              # 1536 edge slots per core
NT = EP // 128                     # 12 edge tiles per core
NCH = EP // 512                    # 3 512-wide chunks for the edge MLP
NU = 1280                          # unique-src node capacity per core
NU_CH = (512, 512, 256)            # node MLP chunking
NB = 768                           # histogram bins; bin NB-1 collects padding
NBC = NB // 128                    # 6 bin chunks
REP = 64                           # inv_cnt replication for dma_gather (256B)
A2T = 4                            # a2 load split (tiles of 4096 free)

f32 = mybir.dt.float32
f16 = mybir.dt.float16
bf16 = mybir.dt.bfloat16
i32 = mybir.dt.int32
i16 = mybir.dt.int16
AF = mybir.ActivationFunctionType
OP = mybir.AluOpType


def _slot_segments(s):
    """(tile, p0, p1) segments of graph slot s in the (p, t) edge grid."""
    segs, a, end = [], s * CAP, (s + 1) * CAP
    while a < end:
        t, p0 = divmod(a, 128)
        take = min(128 - p0, end - a)
        segs.append((t, p0, p0 + take))
        a += take
    return segs


def _emit(nc, tc, io):
    es = ExitStack()
    const = es.enter_context(tc.tile_pool(name="const", bufs=1))
    big = es.enter_context(tc.tile_pool(name="big", bufs=1))
    work = es.enter_context(tc.tile_pool(name="work", bufs=3))
    e3x = es.enter_context(tc.tile_pool(name="e3x", bufs=NT))
    psA = es.enter_context(tc.tile_pool(name="psA", bufs=2, space="PSUM"))
    psB = es.enter_context(tc.tile_pool(name="psB", bufs=3, space="PSUM"))
    psZ = es.enter_context(tc.tile_pool(name="psZ", bufs=2, space="PSUM"))
    psO = es.enter_context(tc.tile_pool(name="psO", bufs=1, space="PSUM"))
    dram = es.enter_context(tc.tile_pool(name="dram", bufs=1, space="DRAM"))

    with es:
        def cload(name, shape, dt=f32):
            t = const.tile(shape, dt, tag=name)
            nc.sync.dma_start(t[:], io[name][:])
            return t

        xuT = cload("xuT", [NODE_DIM, NU])
        idn = cload("ident", [128, 128])
        idn16 = cload("ident16", [128, 128], f16)
        w_p1 = cload("p1w", [NODE_DIM, H])
        b_p1 = cload("p1b", [H, 1])
        w_p2 = cload("p2w", [H, H])
        b_p2 = cload("p2b", [H, 1])
        w_e1 = cload("e1w", [EDGE_DIM, 128])
        b_e1 = cload("e1b", [128, 1])
        w_e2 = cload("e2w", [128, 256])
        b_e2 = cload("e2b", [128, 2])
        w_e30 = const.tile([128, 128], f32, tag="e3w0")
        nc.sync.dma_start(w_e30[:], io["e3w"][0:128, :])
        w_e31 = const.tile([128, 128], f32, tag="e3w1")
        nc.sync.dma_start(w_e31[:], io["e3w"][128:256, :])
        b_e3 = cload("e3b", [128, 1])
        w_br = cload("br", [H, H])
        eaT = cload("eaT", [EDGE_DIM, EP])
        t_src = cload("srcidx", [128, NT], i32)
        t_winv = cload("winv", [128, NT], i32)
        t_dsth = cload("dsth", [128, NT])
        t_mask = cload("mask", [128, NT])

        h_dram = dram.tile([NU, H], f32)
        inv_dram = dram.tile([NB, 1], f32)

        # gpsimd: iota for the histogram (int32 -> fp16)
        ioti = const.tile([128, NB], i32, tag="iotai")
        nc.gpsimd.iota(ioti[:], pattern=[[1, NB]], base=0, channel_multiplier=0)
        iot = const.tile([128, NB], f16, tag="iota")
        nc.vector.tensor_copy(iot[:], ioti[:])

        # ---- node MLP over unique-src nodes (feature-major) -----------------
        n_off = 0
        for cw in NU_CH:
            ps1 = psA.tile([128, 512], f32, tag="mlp")
            nc.tensor.matmul(ps1[:, :cw], w_p1[:], xuT[:, n_off:n_off + cw],
                             start=True, stop=True)
            h1 = work.tile([128, 512], f32, tag="h1")
            nc.scalar.activation(h1[:, :cw], ps1[:, :cw], AF.Relu,
                                 bias=b_p1[:, 0:1])
            ps2 = psA.tile([128, 512], f32, tag="mlp")
            nc.tensor.matmul(ps2[:, :cw], w_p2[:], h1[:, :cw],
                             start=True, stop=True)
            h2 = work.tile([128, 512], f32, tag="h2")
            nc.vector.tensor_scalar_add(h2[:, :cw], ps2[:, :cw], b_p2[:, 0:1])
            for j in range(cw // 128):
                pt = psB.tile([128, 128], f32, tag="tr")
                nc.tensor.transpose(pt[:], h2[:, j * 128:(j + 1) * 128], idn[:])
                hr = work.tile([128, 128], f32, tag="hrow")
                nc.scalar.copy(hr[:], pt[:])
                nc.sync.dma_start(
                    h_dram[n_off + j * 128: n_off + (j + 1) * 128, :], hr[:])
            n_off += cw

        # ---- in-degree histogram (fp16 on DVE) ------------------------------
        oh_acc = big.tile([128, NB], f16, tag="ohacc")
        nc.vector.tensor_scalar(oh_acc[:], iot[:], t_dsth[:, 0:1], None,
                                op0=OP.is_equal)
        for t in range(1, NT):
            oh = work.tile([128, NB], f16, tag="oh")
            nc.vector.tensor_scalar(oh[:], iot[:], t_dsth[:, t:t + 1], None,
                                    op0=OP.is_equal)
            nc.vector.tensor_add(oh_acc[:], oh_acc[:], oh[:])
        cnt = work.tile([128, NBC], f32, tag="cnt")
        for b in range(NBC):
            pt = psB.tile([128, 128], f16, tag="tr")
            nc.tensor.transpose(pt[:], oh_acc[:, b * 128:(b + 1) * 128],
                                idn16[:])
            nc.vector.tensor_reduce(cnt[:, b:b + 1], pt[:],
                                    axis=mybir.AxisListType.X, op=OP.add)
        nc.vector.tensor_scalar_max(cnt[:], cnt[:], 1.0)
        inv = work.tile([128, NBC], f32, tag="inv")
        nc.vector.reciprocal(inv[:], cnt[:])
        nc.sync.dma_start(inv_dram[:, 0].rearrange("(p c) -> p c", c=NBC),
                          inv[:])

        # ---- gathers via SWDGE dma_gather + a2 split loads ------------------
        a2h_sb = big.tile([128, 128 * H], bf16, tag="a2h")
        a2l_sb = big.tile([128, 128 * H], bf16, tag="a2l")
        a2w = 128 * H // A2T
        for q in range(2):
            nc.gpsimd.dma_start(a2h_sb[:, q * a2w:(q + 1) * a2w],
                                io["a2h"][:, q * a2w:(q + 1) * a2w])
        hsrc = big.tile([128, NT, H], f32, tag="hsrc")
        wgath = work.tile([128, NT], f32, tag="wgath")
        for t in range(NT):
            nc.gpsimd.indirect_dma_start(
                out=wgath[:, t:t + 1], out_offset=None, in_=inv_dram[:, :],
                in_offset=bass.IndirectOffsetOnAxis(ap=t_winv[:, t:t + 1],
                                                    axis=0))
            nc.gpsimd.indirect_dma_start(
                out=hsrc[:, t, :], out_offset=None, in_=h_dram[:, :],
                in_offset=bass.IndirectOffsetOnAxis(ap=t_src[:, t:t + 1],
                                                    axis=0))
        for q in range(2, A2T):
            nc.gpsimd.dma_start(a2h_sb[:, q * a2w:(q + 1) * a2w],
                                io["a2h"][:, q * a2w:(q + 1) * a2w])
        for q in range(2):
            nc.gpsimd.dma_start(a2l_sb[:, q * a2w:(q + 1) * a2w],
                                io["a2l"][:, q * a2w:(q + 1) * a2w])
        for q in range(2, A2T):
            nc.gpsimd.dma_start(a2l_sb[:, q * a2w:(q + 1) * a2w],
                                io["a2l"][:, q * a2w:(q + 1) * a2w])
        wme = const.tile([128, NT], f32, tag="wme")
        nc.vector.tensor_tensor(wme[:], wgath[:], t_mask[:], op=OP.mult)

        # ---- edge MLP (feature-major) ---------------------------------------
        e1o = big.tile([128, EP], f32, tag="e1o")
        for q in range(NCH):
            ps = psA.tile([128, 512], f32, tag="mlp")
            nc.tensor.matmul(ps[:], w_e1[:], eaT[:, q * 512:(q + 1) * 512],
                             start=True, stop=True)
            nc.scalar.activation(e1o[:, q * 512:(q + 1) * 512], ps[:], AF.Relu,
                                 bias=b_e1[:, 0:1])
        e2o0 = big.tile([128, EP], f32, tag="e2o0")
        e2o1 = big.tile([128, EP], f32, tag="e2o1")
        for m, e2o in enumerate((e2o0, e2o1)):
            for q in range(NCH):
                ps = psA.tile([128, 512], f32, tag="mlp")
                nc.tensor.matmul(ps[:], w_e2[:, m * 128:(m + 1) * 128],
                                 e1o[:, q * 512:(q + 1) * 512],
                                 start=True, stop=True)
                nc.scalar.activation(e2o[:, q * 512:(q + 1) * 512], ps[:],
                                     AF.Relu, bias=b_e2[:, m:m + 1])
        e3o = big.tile([128, EP], f32, tag="e3o")
        for q in range(NCH):
            ps = psA.tile([128, 512], f32, tag="mlp")
            nc.tensor.matmul(ps[:], w_e30[:], e2o0[:, q * 512:(q + 1) * 512],
                             start=True, stop=False)
            nc.tensor.matmul(ps[:], w_e31[:], e2o1[:, q * 512:(q + 1) * 512],
                             start=False, stop=True)
            nc.scalar.activation(e3o[:, q * 512:(q + 1) * 512], ps[:], AF.Relu,
                                 bias=b_e3[:, 0:1])

        # ---- per-tile transpose to edge-major, scale by w_e -----------------
        e3w_tiles = []
        for t in range(NT):
            pt = psB.tile([128, 128], f32, tag="tr")
            nc.tensor.transpose(pt[:], e3o[:, t * 128:(t + 1) * 128], idn[:])
            ex = e3x.tile([128, H + 1], f32, tag="e3x")
            nc.vector.tensor_scalar_mul(ex[:, 0:H], pt[:], wme[:, t:t + 1])
            nc.vector.tensor_copy(ex[:, H:H + 1], wme[:, t:t + 1])
            e3w_tiles.append(ex)

        # ---- per-graph ZG accumulation + bf16 hi/lo split -------------------
        zg_h = big.tile([128, G_PER_CORE, H], bf16, tag="zgh")
        zg_l = big.tile([128, G_PER_CORE, H], bf16, tag="zgl")
        hw_f = work.tile([128, G_PER_CORE], f32, tag="hwf")
        for s in range(G_PER_CORE):
            segs = _slot_segments(s)
            pz = psZ.tile([128, H + 1], f32, tag="zg")
            for n, (t, p0, p1) in enumerate(segs):
                nc.tensor.matmul(pz[:], hsrc[p0:p1, t, :],
                                 e3w_tiles[t][p0:p1, :],
                                 start=(n == 0), stop=(n == len(segs) - 1))
            zf = work.tile([128, H + 1], f32, tag="zf")
            nc.scalar.copy(zf[:], pz[:])
            nc.vector.tensor_copy(zg_h[:, s, :], zf[:, 0:H])
            zhf = work.tile([128, H], f32, tag="zhf")
            nc.vector.tensor_copy(zhf[:], zg_h[:, s, :])
            nc.vector.tensor_tensor(zg_l[:, s, :], zf[:, 0:H], zhf[:],
                                    op=OP.subtract)
            nc.vector.tensor_copy(hw_f[:, s:s + 1], zf[:, H:H + 1])

        # ---- final e4 contraction: 3-term bf16 split + fp32 bias ------------
        po = psO.tile([128, G_PER_CORE], f32, tag="out")
        first = True
        for ab, zb in ((a2h_sb, zg_h), (a2h_sb, zg_l), (a2l_sb, zg_h)):
            for k in range(H):
                nc.tensor.matmul(po[:], ab[:, k * 128:(k + 1) * 128],
                                 zb[:, :, k], start=first, stop=False)
                first = False
        nc.tensor.matmul(po[:], w_br[:], hw_f[:], start=False, stop=True)
        ot = work.tile([128, G_PER_CORE], f32, tag="ot")
        nc.scalar.copy(ot[:], po[:])
        nc.sync.dma_start(io["pooled_t"][:, :], ot[:])


_CACHE = {}


def _build():
    if "nc" in _CACHE:
        return _CACHE["nc"]
    nc = bacc.Bacc("TRN2", target_bir_lowering=False, debug=False,
                   num_devices=N_CORES)
    io = {}

    def din(name, shape, dt=f32):
        io[name] = nc.dram_tensor(name, shape, dt, kind="ExternalInput").ap()

    din("xuT", [NODE_DIM, NU])
    din("eaT", [EDGE_DIM, EP])
    din("srcidx", [128, NT], i32)
    din("winv", [128, NT], i32)
    din("dsth", [128, NT])
    din("mask", [128, NT])
    din("p1w", [NODE_DIM, H]); din("p1b", [H, 1])
    din("p2w", [H, H]); din("p2b", [H, 1])
    din("e1w", [EDGE_DIM, 128]); din("e1b", [128, 1])
    din("e2w", [128, 256]); din("e2b", [128, 2])
    din("e3w", [256, 128]); din("e3b", [128, 1])
    din("a2h", [128, 128 * H], bf16)
    din("a2l", [128, 128 * H], bf16)
    din("br", [H, H])
    din("ident", [128, 128])
    din("ident16", [128, 128], f16)
    io["pooled_t"] = nc.dram_tensor("pooled_t", [H, G_PER_CORE], f32,
                                    kind="ExternalOutput").ap()

    with tile.TileContext(nc) as tc:
        _emit(nc, tc, io)
    nc.compile()
    _CACHE["nc"] = nc
    return nc


def _host_prep(inputs):
    x = np.ascontiguousarray(np.asarray(inputs["x"], dtype=np.float32))
    ea = np.asarray(inputs["edge_attr"], dtype=np.float32)
    ei = np.asarray(inputs["edge_index"]).astype(np.int64)
    batch = np.asarray(inputs["batch"]).astype(np.int64)
    src, dst = ei[0], ei[1]
    gid = batch[dst]

    a2f = np.ascontiguousarray(
        np.asarray(inputs["e4_w"], np.float32)
        .reshape(128, 128, 128).transpose(1, 0, 2).reshape(128, 128 * H))
    a2h = a2f.astype(ml_dtypes.bfloat16)
    a2l = (a2f - a2h.astype(np.float32)).astype(ml_dtypes.bfloat16)

    com = {
        "p1w": np.asarray(inputs["p1_w"], np.float32),
        "p1b": np.asarray(inputs["p1_b"], np.float32).reshape(H, 1),
        "p2w": np.asarray(inputs["p2_w"], np.float32),
        "p2b": np.asarray(inputs["p2_b"], np.float32).reshape(H, 1),
        "e1w": np.asarray(inputs["e1_w"], np.float32),
        "e1b": np.asarray(inputs["e1_b"], np.float32).reshape(128, 1),
        "e2w": np.asarray(inputs["e2_w"], np.float32),
        "e2b": np.ascontiguousarray(
            np.asarray(inputs["e2_b"], np.float32).reshape(2, 128).T),
        "e3w": np.asarray(inputs["e3_w"], np.float32),
        "e3b": np.asarray(inputs["e3_b"], np.float32).reshape(128, 1),
        "a2h": a2h, "a2l": a2l,
        "br": np.ascontiguousarray(
            np.asarray(inputs["e4_b"], np.float32).reshape(128, 128)),
        "ident": np.eye(128, dtype=np.float32),
        "ident16": np.eye(128, dtype=np.float16),
    }
    com = {k: np.ascontiguousarray(v) for k, v in com.items()}

    ns = np.searchsorted(batch, np.arange(0, G + 1, G_PER_CORE))
    in_maps = []
    for c in range(N_CORES):
        n0, n1 = int(ns[c]), int(ns[c + 1])
        assert n1 - n0 <= NB - 2, f"core {c} has {n1 - n0} nodes > {NB - 2}"
        ea_s = np.zeros((EP, EDGE_DIM), np.float32)
        srcg = np.zeros(EP, np.int64)
        dstl_s = np.full(EP, NB - 1, np.int64)
        mask_s = np.zeros(EP, np.float32)
        filled = np.zeros(EP, bool)
        for s in range(G_PER_CORE):
            es = np.where(gid == c * G_PER_CORE + s)[0]
            assert len(es) <= CAP, f"graph {c * G_PER_CORE + s}: {len(es)} edges"
            pos = s * CAP + np.arange(len(es))
            ea_s[pos] = ea[es]
            srcg[pos] = src[es]
            dstl_s[pos] = dst[es] - n0
            mask_s[pos] = 1.0
            filled[pos] = True
        uniq = np.unique(srcg[filled])
        assert len(uniq) <= NU, f"core {c}: {len(uniq)} unique srcs > {NU}"
        srcl = np.searchsorted(uniq, srcg)
        srcl[~filled] = 0
        xu = np.zeros((NU, NODE_DIM), np.float32)
        xu[:len(uniq)] = x[uniq]

        def grid(a, dt):  # slot pos = t*128 + p  ->  [p, t]
            return np.ascontiguousarray(a.reshape(NT, 128).T.astype(dt))

        m = dict(com)
        m["xuT"] = np.ascontiguousarray(xu.T)
        m["eaT"] = np.ascontiguousarray(ea_s.T)
        winv_s = (dstl_s % 128) * NBC + dstl_s // 128
        m["srcidx"] = grid(srcl, np.int32)
        m["winv"] = grid(winv_s, np.int32)
        m["dsth"] = grid(dstl_s, np.float32)
        m["mask"] = grid(mask_s, np.float32)
        in_maps.append(m)
    return in_maps


def _run(inputs, trace=False, tmpdir=None):
    nc = _build()
    in_maps = _host_prep(inputs)
    if trace:
        # No egress in this sandbox: neutralize the artifact upload the
        # trace path performs after NTFF capture, and register the NTFF
        # hook module if the image lacks antenv.axon_hooks.
        from concourse import bass_utils as _bu
        _bu.upload_artifacts = lambda d: d
        try:
            from antenv import axon_hooks  # noqa: F401
        except ImportError:
            import importlib.util, sys as _sys
            spec = importlib.util.spec_from_file_location(
                "antenv.axon_hooks", "/opt/trn_rl_repo/antenv/axon_hooks.py")
            mod = importlib.util.module_from_spec(spec)
            spec.loader.exec_module(mod)
            _sys.modules["antenv.axon_hooks"] = mod
    res = run_bass_kernel_spmd(nc, in_maps, list(range(N_CORES)),
                               trace=trace, tmpdir=tmpdir)
    out = np.empty((G, H), np.float32)
    for c in range(N_CORES):
        out[c * G_PER_CORE:(c + 1) * G_PER_CORE, :] = res.results[c]["pooled_t"].T
    return out, res


def kernel(**inputs) -> np.ndarray:
    out, _ = _run(inputs)
    return out


# revision 13
# speedup vs baseline: 2.1217x; 1.2782x over previous
"""Trainium2 Bass kernel for nn_Net_16174846837292 (NNConv GNN message passing).

Strategy (graph-sharded, aggregation-folded):
  pooled[g,o] = sum_{e: batch[dst[e]]=g} w_e * msg[e,o],  w_e = 1/max(cnt[dst_e],1)
  msg[e,o]    = sum_{k,i} e3[e,k]*h[src_e,i]*e4w[k,i*128+o] + sum_i h[src_e,i]*e4b[i*128+o]
  => pooled^T[o,g] = sum_k A2[k*128+i, o]^T ZG_g[i,k] + Br^T HW_g        (tiny matmuls)
     ZG_g[i,k] = sum_{e in g} w_e h[src_e,i] e3[e,k],  HW_g[i] = sum_{e in g} w_e h[src_e,i]
  Never materializes the per-edge [128,128] weight matrices (512 MB in the
  reference) nor any per-node [16384] intermediates.

Sharding: edges grouped by the graph of their destination node; 8 graphs per
core. Each graph's incoming edges live entirely on one core, so in-degree
counts are core-local and NO collectives are needed. Per-core edges pack into
8 slots of 192 (64-aligned segments for the per-graph PSUM accumulation).
Node MLP runs only over the <=1280 unique src nodes each core references.
The e4 contraction runs as a 3-term bf16 split (zh@ah + zl@ah + zh@al, fp32
PSUM) — max error vs fp32 measured at 2.9e-6 of output scale.
"""

import numpy as np
from contextlib import ExitStack

import ml_dtypes
import concourse.bass as bass
import concourse.tile as tile
from concourse import bacc, mybir
from concourse.bass_utils import run_bass_kernel_spmd

N_CORES = 8
N, E, G, H = 4096, 8192, 64, 128
NODE_DIM, EDGE_DIM = 11, 5
G_PER_CORE = G // N_CORES          # 8 graph slots per core
CAP = 192                          # edge slots per graph (64-aligned segments)
EP = G_PER_CORE * CAP              # 1536 edge slots per core
NT = EP // 128                     # 12 edge tiles per core
NCH = EP // 512                    # 3 512-wide chunks for the edge MLP
NU = 1280                          # unique-src node capacity per core
NU_CH = (512, 512, 256)            # node MLP chunking
NB = 768                           # histogram bins; bin NB-1 collects padding
NBC = NB // 128                    # 6 bin chunks
REP = 64                           # inv_cnt replication for dma_gather (256B)
A2T = 4                            # a2 load split (tiles of 4096 free)

f32 = mybir.dt.float32
f16 = mybir.dt.float16
bf16 = mybir.dt.bfloat16
i32 = mybir.dt.int32
i16 = mybir.dt.int16
AF = mybir.ActivationFunctionType
OP = mybir.AluOpType


def _slot_segments(s):
    """(tile, p0, p1) segments of graph slot s in the (p, t) edge grid."""
    segs, a, end = [], s * CAP, (s + 1) * CAP
    while a < end:
        t, p0 = divmod(a, 128)
        take = min(128 - p0, end - a)
        segs.append((t, p0, p0 + take))
        a += take
    return segs


def _emit(nc, tc, io):
    es = ExitStack()
    const = es.enter_context(tc.tile_pool(name="const", bufs=1))
    big = es.enter_context(tc.tile_pool(name="big", bufs=1))
    work = es.enter_context(tc.tile_pool(name="work", bufs=3))
    e3x = es.enter_context(tc.tile_pool(name="e3x", bufs=NT))
    psA = es.enter_context(tc.tile_pool(name="psA", bufs=3, space="PSUM"))
    psB = es.enter_context(tc.tile_pool(name="psB", bufs=2, space="PSUM"))
    psZ = es.enter_context(tc.tile_pool(name="psZ", bufs=2, space="PSUM"))
    psO = es.enter_context(tc.tile_pool(name="psO", bufs=1, space="PSUM"))
    dram = es.enter_context(tc.tile_pool(name="dram", bufs=1, space="DRAM"))

    with es:
        def cload(name, shape, dt=f32):
            t = const.tile(shape, dt, tag=name)
            nc.sync.dma_start(t[:], io[name][:])
            return t

        xuT = cload("xuT", [NODE_DIM, NU])
        t_dsth = cload("dsth", [128, NT])
        idn16 = cload("ident16", [128, 128], f16)
        idn = cload("ident", [128, 128])
        w_p1 = cload("p1w", [NODE_DIM, H])
        b_p1 = cload("p1b", [H, 1])
        w_p2 = cload("p2w", [H, H])
        b_p2 = cload("p2b", [H, 1])
        w_e1 = cload("e1w", [EDGE_DIM, 128])
        b_e1 = cload("e1b", [128, 1])
        w_e2 = cload("e2w", [128, 256])
        b_e2 = cload("e2b", [128, 2])
        w_e30 = const.tile([128, 128], f32, tag="e3w0")
        nc.sync.dma_start(w_e30[:], io["e3w"][0:128, :])
        w_e31 = const.tile([128, 128], f32, tag="e3w1")
        nc.sync.dma_start(w_e31[:], io["e3w"][128:256, :])
        b_e3 = cload("e3b", [128, 1])
        w_br = cload("br", [H, H])
        eaT = cload("eaT", [EDGE_DIM, EP])
        t_src = cload("srcidx", [128, NT], i32)
        t_winv = cload("winv", [128, NT], i32)
        t_mask = cload("mask", [128, NT])

        h_dram = dram.tile([NU, H], f32)
        inv_dram = dram.tile([NB, 1], f32)

        # gpsimd: iota for the histogram (int32 -> fp16)
        ioti = const.tile([128, NB], i32, tag="iotai")
        nc.gpsimd.iota(ioti[:], pattern=[[1, NB]], base=0, channel_multiplier=0)
        iot = const.tile([128, NB], f16, tag="iota")
        nc.vector.tensor_copy(iot[:], ioti[:])

        # ---- in-degree histogram (fp16 on DVE) ------------------------------
        oh_acc = big.tile([128, NB], f16, tag="ohacc")
        nc.vector.tensor_scalar(oh_acc[:], iot[:], t_dsth[:, 0:1], None,
                                op0=OP.is_equal)
        for t in range(1, NT):
            oh = work.tile([128, NB], f16, tag="oh")
            nc.vector.tensor_scalar(oh[:], iot[:], t_dsth[:, t:t + 1], None,
                                    op0=OP.is_equal)
            nc.vector.tensor_add(oh_acc[:], oh_acc[:], oh[:])
        cnt = work.tile([128, NBC], f32, tag="cnt")
        for b in range(NBC):
            pt = psB.tile([128, 128], f16, tag="tr")
            nc.tensor.transpose(pt[:], oh_acc[:, b * 128:(b + 1) * 128],
                                idn16[:])
            nc.vector.tensor_reduce(cnt[:, b:b + 1], pt[:],
                                    axis=mybir.AxisListType.X, op=OP.add)
        nc.vector.tensor_scalar_max(cnt[:], cnt[:], 1.0)
        inv = work.tile([128, NBC], f32, tag="inv")
        nc.vector.reciprocal(inv[:], cnt[:])
        nc.sync.dma_start(inv_dram[:, 0].rearrange("(p c) -> p c", c=NBC),
                          inv[:])

        # ---- node MLP over unique-src nodes (feature-major) -----------------
        n_off = 0
        for cw in NU_CH:
            ps1 = psA.tile([128, 512], f32, tag="mlp")
            nc.tensor.matmul(ps1[:, :cw], w_p1[:], xuT[:, n_off:n_off + cw],
                             start=True, stop=True)
            h1 = work.tile([128, 512], f32, tag="h1")
            nc.scalar.activation(h1[:, :cw], ps1[:, :cw], AF.Relu,
                                 bias=b_p1[:, 0:1])
            ps2 = psA.tile([128, 512], f32, tag="mlp")
            nc.tensor.matmul(ps2[:, :cw], w_p2[:], h1[:, :cw],
                             start=True, stop=True)
            h2 = work.tile([128, 512], f32, tag="h2")
            nc.vector.tensor_scalar_add(h2[:, :cw], ps2[:, :cw], b_p2[:, 0:1])
            for j in range(cw // 128):
                pt = psB.tile([128, 128], f32, tag="tr")
                nc.tensor.transpose(pt[:], h2[:, j * 128:(j + 1) * 128], idn[:])
                hr = work.tile([128, 128], f32, tag="hrow")
                nc.scalar.copy(hr[:], pt[:])
                nc.sync.dma_start(
                    h_dram[n_off + j * 128: n_off + (j + 1) * 128, :], hr[:])
            n_off += cw

        # ---- per-edge gathers (SWDGE) + a2 loads (HWDGE) -------------------
        a2h_sb = big.tile([128, 128 * H], bf16, tag="a2h")
        a2l_sb = big.tile([128, 128 * H], bf16, tag="a2l")
        a2w = 128 * H // A2T
        for q in range(A2T):
            nc.sync.dma_start(a2h_sb[:, q * a2w:(q + 1) * a2w],
                              io["a2h"][:, q * a2w:(q + 1) * a2w])
        for q in range(A2T):
            nc.sync.dma_start(a2l_sb[:, q * a2w:(q + 1) * a2w],
                              io["a2l"][:, q * a2w:(q + 1) * a2w])
        hsrc = big.tile([128, NT, H], f32, tag="hsrc")
        wgath = work.tile([128, NT], f32, tag="wgath")
        for t in range(NT):
            nc.gpsimd.indirect_dma_start(
                out=wgath[:, t:t + 1], out_offset=None, in_=inv_dram[:, :],
                in_offset=bass.IndirectOffsetOnAxis(ap=t_winv[:, t:t + 1],
                                                    axis=0))
        for t in range(NT):
            nc.gpsimd.indirect_dma_start(
                out=hsrc[:, t, :], out_offset=None, in_=h_dram[:, :],
                in_offset=bass.IndirectOffsetOnAxis(ap=t_src[:, t:t + 1],
                                                    axis=0))
        wme = const.tile([128, NT], f32, tag="wme")
        nc.vector.tensor_tensor(wme[:], wgath[:], t_mask[:], op=OP.mult)

        # ---- edge MLP (feature-major) ---------------------------------------
        e1o = big.tile([128, EP], f32, tag="e1o")
        for q in range(NCH):
            ps = psA.tile([128, 512], f32, tag="mlp")
            nc.tensor.matmul(ps[:], w_e1[:], eaT[:, q * 512:(q + 1) * 512],
                             start=True, stop=True)
            nc.scalar.activation(e1o[:, q * 512:(q + 1) * 512], ps[:], AF.Relu,
                                 bias=b_e1[:, 0:1])
        e2o0 = big.tile([128, EP], f32, tag="e2o0")
        e2o1 = big.tile([128, EP], f32, tag="e2o1")
        for m, e2o in enumerate((e2o0, e2o1)):
            for q in range(NCH):
                ps = psA.tile([128, 512], f32, tag="mlp")
                nc.tensor.matmul(ps[:], w_e2[:, m * 128:(m + 1) * 128],
                                 e1o[:, q * 512:(q + 1) * 512],
                                 start=True, stop=True)
                nc.scalar.activation(e2o[:, q * 512:(q + 1) * 512], ps[:],
                                     AF.Relu, bias=b_e2[:, m:m + 1])
        e3o = big.tile([128, EP], f32, tag="e3o")
        for q in range(NCH):
            ps = psA.tile([128, 512], f32, tag="mlp")
            nc.tensor.matmul(ps[:], w_e30[:], e2o0[:, q * 512:(q + 1) * 512],
                             start=True, stop=False)
            nc.tensor.matmul(ps[:], w_e31[:], e2o1[:, q * 512:(q + 1) * 512],
                             start=False, stop=True)
            nc.scalar.activation(e3o[:, q * 512:(q + 1) * 512], ps[:], AF.Relu,
                                 bias=b_e3[:, 0:1])

        # ---- per-tile transpose to edge-major, scale by w_e -----------------
        e3w_tiles = []
        for t in range(NT):
            pt = psB.tile([128, 128], f32, tag="tr")
            nc.tensor.transpose(pt[:], e3o[:, t * 128:(t + 1) * 128], idn[:])
            ex = e3x.tile([128, H + 1], f32, tag="e3x")
            nc.vector.tensor_scalar_mul(ex[:, 0:H], pt[:], wme[:, t:t + 1])
            nc.vector.tensor_copy(ex[:, H:H + 1], wme[:, t:t + 1])
            e3w_tiles.append(ex)

        # ---- per-graph ZG accumulation + bf16 hi/lo split -------------------
        zg_h = big.tile([128, G_PER_CORE, H], bf16, tag="zgh")
        zg_l = big.tile([128, G_PER_CORE, H], bf16, tag="zgl")
        hw_f = work.tile([128, G_PER_CORE], f32, tag="hwf")
        for s in range(G_PER_CORE):
            segs = _slot_segments(s)
            pz = psZ.tile([128, H + 1], f32, tag="zg")
            for n, (t, p0, p1) in enumerate(segs):
                nc.tensor.matmul(pz[:], hsrc[p0:p1, t, :],
                                 e3w_tiles[t][p0:p1, :],
                                 start=(n == 0), stop=(n == len(segs) - 1))
            zf = work.tile([128, H + 1], f32, tag="zf")
            nc.scalar.copy(zf[:], pz[:])
            nc.vector.tensor_copy(zg_h[:, s, :], zf[:, 0:H])
            zhf = work.tile([128, H], f32, tag="zhf")
            nc.vector.tensor_copy(zhf[:], zg_h[:, s, :])
            nc.vector.tensor_tensor(zg_l[:, s, :], zf[:, 0:H], zhf[:],
                                    op=OP.subtract)
            nc.vector.tensor_copy(hw_f[:, s:s + 1], zf[:, H:H + 1])

        # ---- final e4 contraction: 3-term bf16 split + fp32 bias ------------
        po = psO.tile([128, G_PER_CORE], f32, tag="out")
        first = True
        for ab, zb in ((a2h_sb, zg_h), (a2h_sb, zg_l), (a2l_sb, zg_h)):
            for k in range(H):
                nc.tensor.matmul(po[:], ab[:, k * 128:(k + 1) * 128],
                                 zb[:, :, k], start=first, stop=False)
                first = False
        nc.tensor.matmul(po[:], w_br[:], hw_f[:], start=False, stop=True)
        ot = work.tile([128, G_PER_CORE], f32, tag="ot")
        nc.scalar.copy(ot[:], po[:])
        nc.sync.dma_start(io["pooled_t"][:, :], ot[:])


_CACHE = {}


def _build():
    if "nc" in _CACHE:
        return _CACHE["nc"]
    nc = bacc.Bacc("TRN2", target_bir_lowering=False, debug=False,
                   num_devices=N_CORES)
    io = {}

    def din(name, shape, dt=f32):
        io[name] = nc.dram_tensor(name, shape, dt, kind="ExternalInput").ap()

    din("xuT", [NODE_DIM, NU])
    din("eaT", [EDGE_DIM, EP])
    din("srcidx", [128, NT], i32)
    din("winv", [128, NT], i32)
    din("dsth", [128, NT])
    din("mask", [128, NT])
    din("p1w", [NODE_DIM, H]); din("p1b", [H, 1])
    din("p2w", [H, H]); din("p2b", [H, 1])
    din("e1w", [EDGE_DIM, 128]); din("e1b", [128, 1])
    din("e2w", [128, 256]); din("e2b", [128, 2])
    din("e3w", [256, 128]); din("e3b", [128, 1])
    din("a2h", [128, 128 * H], bf16)
    din("a2l", [128, 128 * H], bf16)
    din("br", [H, H])
    din("ident", [128, 128])
    din("ident16", [128, 128], f16)
    io["pooled_t"] = nc.dram_tensor("pooled_t", [H, G_PER_CORE], f32,
                                    kind="ExternalOutput").ap()

    with tile.TileContext(nc) as tc:
        _emit(nc, tc, io)
    nc.compile()
    _CACHE["nc"] = nc
    return nc


def _host_prep(inputs):
    x = np.ascontiguousarray(np.asarray(inputs["x"], dtype=np.float32))
    ea = np.asarray(inputs["edge_attr"], dtype=np.float32)
    ei = np.asarray(inputs["edge_index"]).astype(np.int64)
    batch = np.asarray(inputs["batch"]).astype(np.int64)
    src, dst = ei[0], ei[1]
    gid = batch[dst]

    a2f = np.ascontiguousarray(
        np.asarray(inputs["e4_w"], np.float32)
        .reshape(128, 128, 128).transpose(1, 0, 2).reshape(128, 128 * H))
    a2h = a2f.astype(ml_dtypes.bfloat16)
    a2l = (a2f - a2h.astype(np.float32)).astype(ml_dtypes.bfloat16)

    com = {
        "p1w": np.asarray(inputs["p1_w"], np.float32),
        "p1b": np.asarray(inputs["p1_b"], np.float32).reshape(H, 1),
        "p2w": np.asarray(inputs["p2_w"], np.float32),
        "p2b": np.asarray(inputs["p2_b"], np.float32).reshape(H, 1),
        "e1w": np.asarray(inputs["e1_w"], np.float32),
        "e1b": np.asarray(inputs["e1_b"], np.float32).reshape(128, 1),
        "e2w": np.asarray(inputs["e2_w"], np.float32),
        "e2b": np.ascontiguousarray(
            np.asarray(inputs["e2_b"], np.float32).reshape(2, 128).T),
        "e3w": np.asarray(inputs["e3_w"], np.float32),
        "e3b": np.asarray(inputs["e3_b"], np.float32).reshape(128, 1),
        "a2h": a2h, "a2l": a2l,
        "br": np.ascontiguousarray(
            np.asarray(inputs["e4_b"], np.float32).reshape(128, 128)),
        "ident": np.eye(128, dtype=np.float32),
        "ident16": np.eye(128, dtype=np.float16),
    }
    com = {k: np.ascontiguousarray(v) for k, v in com.items()}

    ns = np.searchsorted(batch, np.arange(0, G + 1, G_PER_CORE))
    in_maps = []
    for c in range(N_CORES):
        n0, n1 = int(ns[c]), int(ns[c + 1])
        assert n1 - n0 <= NB - 2, f"core {c} has {n1 - n0} nodes > {NB - 2}"
        ea_s = np.zeros((EP, EDGE_DIM), np.float32)
        srcg = np.zeros(EP, np.int64)
        dstl_s = np.full(EP, NB - 1, np.int64)
        mask_s = np.zeros(EP, np.float32)
        filled = np.zeros(EP, bool)
        for s in range(G_PER_CORE):
            es = np.where(gid == c * G_PER_CORE + s)[0]
            assert len(es) <= CAP, f"graph {c * G_PER_CORE + s}: {len(es)} edges"
            pos = s * CAP + np.arange(len(es))
            ea_s[pos] = ea[es]
            srcg[pos] = src[es]
            dstl_s[pos] = dst[es] - n0
            mask_s[pos] = 1.0
            filled[pos] = True
        uniq = np.unique(srcg[filled])
        assert len(uniq) <= NU, f"core {c}: {len(uniq)} unique srcs > {NU}"
        srcl = np.searchsorted(uniq, srcg)
        srcl[~filled] = 0
        xu = np.zeros((NU, NODE_DIM), np.float32)
        xu[:len(uniq)] = x[uniq]

        def grid(a, dt):  # slot pos = t*128 + p  ->  [p, t]
            return np.ascontiguousarray(a.reshape(NT, 128).T.astype(dt))

        m = dict(com)
        m["xuT"] = np.ascontiguousarray(xu.T)
        m["eaT"] = np.ascontiguousarray(ea_s.T)
        winv_s = (dstl_s % 128) * NBC + dstl_s // 128
        m["srcidx"] = grid(srcl, np.int32)
        m["winv"] = grid(winv_s, np.int32)
        m["dsth"] = grid(dstl_s, np.float32)
        m["mask"] = grid(mask_s, np.float32)
        in_maps.append(m)
    return in_maps


def _run(inputs, trace=False, tmpdir=None):
    nc = _build()
    in_maps = _host_prep(inputs)
    if trace:
        # No egress in this sandbox: neutralize the artifact upload the
        # trace path performs after NTFF capture, and register the NTFF
        # hook module if the image lacks antenv.axon_hooks.
        from concourse import bass_utils as _bu
        _bu.upload_artifacts = lambda d: d
        try:
            from antenv import axon_hooks  # noqa: F401
        except ImportError:
            import importlib.util, sys as _sys
            spec = importlib.util.spec_from_file_location(
                "antenv.axon_hooks", "/opt/trn_rl_repo/antenv/axon_hooks.py")
            mod = importlib.util.module_from_spec(spec)
            spec.loader.exec_module(mod)
            _sys.modules["antenv.axon_hooks"] = mod
    res = run_bass_kernel_spmd(nc, in_maps, list(range(N_CORES)),
                               trace=trace, tmpdir=tmpdir)
    out = np.empty((G, H), np.float32)
    for c in range(N_CORES):
        out[c * G_PER_CORE:(c + 1) * G_PER_CORE, :] = res.results[c]["pooled_t"].T
    return out, res


def kernel(**inputs) -> np.ndarray:
    out, _ = _run(inputs)
    return out
